# revision 1
# baseline (speedup 1.0000x reference)
"""Cross-attention kernel for Trainium2, 8 NeuronCores, data-parallel over batch.

Problem (per batch element b, one per core):
    q  = x_b @ Wq.T + bq                      [T=1024, C=1024]
    kv = enc_b @ Wkv.T + bkv                  [I=576, 2C]
    per head h (H=16, D=64):
        att = softmax((q_h @ k_h.T) / sqrt(D))
        y_h = att @ v_h
    out = y @ Wo.T + bo                       [T, C]

Design notes:
  - One batch element per core (B=8 == n_cores), no collectives.
  - Weights are pre-transposed on host to [in, out] layout so the
    contraction dim (c) lands on SBUF partitions for matmuls.
  - x / enc are transposed on-device via the PE (out = in.T @ I).
  - Matmuls run as float32r (TF32-like, 1 cyc/row at N>=256) via AP bitcast.
  - Attention is computed in S^T = K_h @ Q_h^T orientation ([i, t]); exp is
    applied without max-subtraction (scores are O(1), exp <= ~e^6).  The
    softmax denominator Z_t falls out of the AV matmul by augmenting V with
    a ones column (lhsT M=65); normalization multiplies y^T by a rank-1
    PE-broadcast of 1/Z.
  - Biases: bq/bk are per-partition adds; bv/bo are rank-1 (K=1) matmul
    accumulates of ones^T (x) bias_row.
"""

import numpy as np

T = 1024
C = 1024
I = 576
H = 16
D = 64
NCC = C // 128          # 8 contraction chunks
NIC = (I + 127) // 128  # 5 i chunks (128,128,128,128,64)
I_CH = [128, 128, 128, 128, 64]
VW = 68                 # per-head column block in V tile: 64 v cols + ones col + pad
SCALE = 1.0 / np.sqrt(D)

_CACHE = {}


def _build_nc():
    import concourse.bass as bass
    import concourse.bacc as bacc
    import concourse.mybir as mybir
    import concourse.tile as tile
    from contextlib import ExitStack

    f32 = mybir.dt.float32
    f32r = mybir.dt.float32r

    nc = bacc.Bacc()

    x_d = nc.dram_tensor("x", [T, C], f32r, kind="ExternalInput")
    enc_d = nc.dram_tensor("enc", [I, C], f32r, kind="ExternalInput")
    wqT_d = nc.dram_tensor("wqT", [C, C], f32r, kind="ExternalInput")
    wkT_d = nc.dram_tensor("wkT", [C, C], f32r, kind="ExternalInput")
    wvT_d = nc.dram_tensor("wvT", [C, C], f32r, kind="ExternalInput")
    woT_d = nc.dram_tensor("woT", [C, C], f32r, kind="ExternalInput")
    bq_d = nc.dram_tensor("bq", [C], f32, kind="ExternalInput")
    bk_d = nc.dram_tensor("bk", [C], f32, kind="ExternalInput")
    bv_d = nc.dram_tensor("bv", [C], f32r, kind="ExternalInput")
    bo_d = nc.dram_tensor("bo", [C], f32r, kind="ExternalInput")
    out_d = nc.dram_tensor("out", [T, C], f32, kind="ExternalOutput")

    with ExitStack() as ctx:
        tc = ctx.enter_context(tile.TileContext(nc))

        # long-lived pools
        resid = ctx.enter_context(tc.tile_pool(name="resid", bufs=1))
        misc = ctx.enter_context(tc.tile_pool(name="misc", bufs=1))
        pa = ctx.enter_context(tc.tile_pool(name="pa", bufs=6, space="PSUM"))
        exps = ctx.enter_context(tc.tile_pool(name="exps", bufs=10))

        # constants (DMA'd from NEFF-embedded data; engines can't memset f32r)
        ident_d = nc.inline_tensor(np.eye(128, dtype=np.float32), name="ident_d")
        ones_d = nc.inline_tensor(np.ones((128, 128), dtype=np.float32), name="ones_d")
        ident = misc.tile([128, 128], f32r)
        nc.sync.dma_start(out=ident, in_=ident_d[:, :].bitcast(f32r))
        ones_t = misc.tile([128, 128], f32r)
        nc.sync.dma_start(out=ones_t, in_=ones_d[:, :].bitcast(f32r))
        bq_t = misc.tile([128, NCC], f32)
        nc.sync.dma_start(out=bq_t, in_=bq_d[:].rearrange("(oc p) -> p oc", p=128))
        bk_t = misc.tile([128, NCC], f32)
        nc.sync.dma_start(out=bk_t, in_=bk_d[:].rearrange("(oc p) -> p oc", p=128))
        bv_row = misc.tile([1, C], f32r)
        nc.sync.dma_start(out=bv_row, in_=bv_d[:].unsqueeze(0))
        bo_row = misc.tile([1, C], f32r)
        nc.sync.dma_start(out=bo_row, in_=bo_d[:].unsqueeze(0))

        # resident tensors
        QT = [resid.tile([128, T], f32r, tag=f"QT{i}", name=f"QT{i}") for i in range(NCC)]
        KT = [resid.tile([128, I], f32r, tag=f"KT{i}", name=f"KT{i}") for i in range(NCC)]
        V3 = [resid.tile([128, H, VW], f32r, tag=f"V{i}", name=f"V{i}") for i in range(NIC)]
        YT = [resid.tile([128, T], f32r, tag=f"YT{i}", name=f"YT{i}") for i in range(NCC)]

        with tc.tile_pool(name="ph1", bufs=1) as ph1, \
             tc.tile_pool(name="xin", bufs=3) as xin, \
             tc.tile_pool(name="wsm", bufs=4) as wsm, \
             tc.tile_pool(name="wv8", bufs=1) as wv8, \
             tc.tile_pool(name="pt", bufs=2, space="PSUM") as pt:

            # ---- enc^T (resident through V proj) ----
            encT = [ph1.tile([128, I], f32r, tag=f"encT{i}", name=f"encT{i}") for i in range(NCC)]
            for ii in range(NIC):
                pi = I_CH[ii]
                e_nat = xin.tile([128, C], f32r, tag="xin")
                nc.sync.dma_start(out=e_nat[:pi], in_=enc_d[ii * 128 : ii * 128 + pi])
                for cc in range(NCC):
                    ps = pt.tile([128, 128], f32r, tag="pt")
                    nc.tensor.transpose(
                        ps[:128, :pi],
                        e_nat[:pi, cc * 128 : (cc + 1) * 128],
                        ident[:pi, :pi],
                    )
                    nc.vector.tensor_copy(
                        encT[cc][:, ii * 128 : ii * 128 + pi], ps[:128, :pi]
                    )

            # ---- x^T in t-halves + Q^T projection ----
            for tch in range(2):
                xTh = [ph1.tile([128, 512], f32r, tag=f"xTh{i}", name=f"xTh{i}") for i in range(NCC)]
                for ts in range(4):
                    tt = tch * 4 + ts
                    x_nat = xin.tile([128, C], f32r, tag="xin")
                    nc.sync.dma_start(out=x_nat, in_=x_d[tt * 128 : (tt + 1) * 128])
                    for cc in range(NCC):
                        ps = pt.tile([128, 128], f32r, tag="pt")
                        nc.tensor.transpose(
                            ps, x_nat[:, cc * 128 : (cc + 1) * 128], ident
                        )
                        nc.vector.tensor_copy(
                            xTh[cc][:, ts * 128 : (ts + 1) * 128], ps
                        )
                # Q^T[o, t-half] = (WqT).T @ x^T ; accumulate over c chunks
                for oc in range(NCC):
                    pq = pa.tile([128, 512], f32, tag="pa")
                    for cc in range(NCC):
                        wch = wsm.tile([128, 128], f32r, tag="wsm")
                        nc.sync.dma_start(
                            out=wch,
                            in_=wqT_d[
                                cc * 128 : (cc + 1) * 128, oc * 128 : (oc + 1) * 128
                            ],
                        )
                        nc.tensor.matmul(
                            pq,
                            wch,
                            xTh[cc],
                            start=(cc == 0),
                            stop=(cc == NCC - 1),
                        )
                    nc.vector.tensor_scalar_add(
                        QT[oc][:, tch * 512 : (tch + 1) * 512],
                        pq,
                        bq_t[:, oc : oc + 1],
                    )

            # ---- K^T projection (i in halves of 288) ----
            for oc in range(NCC):
                pk = [pa.tile([128, 288], f32, tag="pa", name=f"pk{_}") for _ in range(2)]
                for cc in range(NCC):
                    wch = wsm.tile([128, 128], f32r, tag="wsm")
                    nc.sync.dma_start(
                        out=wch,
                        in_=wkT_d[
                            cc * 128 : (cc + 1) * 128, oc * 128 : (oc + 1) * 128
                        ],
                    )
                    for ih in range(2):
                        nc.tensor.matmul(
                            pk[ih],
                            wch,
                            encT[cc][:, ih * 288 : (ih + 1) * 288],
                            start=(cc == 0),
                            stop=(cc == NCC - 1),
                        )
                for ih in range(2):
                    nc.vector.tensor_scalar_add(
                        KT[oc][:, ih * 288 : (ih + 1) * 288],
                        pk[ih],
                        bk_t[:, oc : oc + 1],
                    )

            # ---- V projection into [128, H, VW] layout with ones columns ----
            for ii in range(NIC):
                # ones column (head-block col 64) for the fused Z row in AV
                nc.sync.dma_start(
                    out=V3[ii][:, :, 64:65],
                    in_=ones_d[:, 0:H].bitcast(f32r).unsqueeze(2),
                )
            for och in range(2):
                wvt = [wv8.tile([128, 512], f32r, tag=f"wv{i}", name=f"wv{i}") for i in range(NCC)]
                for cc in range(NCC):
                    nc.sync.dma_start(
                        out=wvt[cc],
                        in_=wvT_d[cc * 128 : (cc + 1) * 128, och * 512 : (och + 1) * 512],
                    )
                for ii in range(NIC):
                    pi = I_CH[ii]
                    pv = pa.tile([128, 512], f32, tag="pa")
                    for cc in range(NCC):
                        nc.tensor.matmul(
                            pv[:pi],
                            encT[cc][:, ii * 128 : ii * 128 + pi],
                            wvt[cc],
                            start=(cc == 0),
                            stop=False,
                        )
                    # bv: rank-1 ones^T (x) bv_row accumulate
                    nc.tensor.matmul(
                        pv[:pi],
                        ones_t[0:1, :pi],
                        bv_row[0:1, och * 512 : (och + 1) * 512],
                        start=False,
                        stop=True,
                    )
                    dst = V3[ii][:pi, och * 8 : och * 8 + 8, 0:64]
                    nc.vector.tensor_copy(
                        dst, pv[:pi].rearrange("p (h d) -> p h d", d=64)
                    )

        # ---- attention ----
        with tc.tile_pool(name="attn", bufs=3) as attn:
            for h in range(H):
                oc = h // 2
                hb = (h % 2) * 64
                for tch in range(2):
                    tsl = slice(tch * 512, (tch + 1) * 512)
                    # S^T chunks -> exp -> sbuf
                    es = []
                    for ii in range(NIC):
                        pi = I_CH[ii]
                        ps = pa.tile([128, 512], f32, tag="pa")
                        nc.tensor.matmul(
                            ps[:pi],
                            KT[oc][hb : hb + 64, ii * 128 : ii * 128 + pi],
                            QT[oc][hb : hb + 64, tsl],
                            start=True,
                            stop=True,
                        )
                        e = exps.tile([128, 512], f32r, tag="exps")
                        nc.scalar.activation(
                            e[:pi],
                            ps[:pi],
                            mybir.ActivationFunctionType.Exp,
                            scale=float(SCALE),
                        )
                        es.append(e)
                    # y^T (64 rows) and Z (row 64) via V augmented with ones col
                    py = pa.tile([128, 512], f32, tag="pa")
                    for ii in range(NIC):
                        pi = I_CH[ii]
                        nc.tensor.matmul(
                            py[:65],
                            V3[ii][:pi, h, 0:65],
                            es[ii][:pi],
                            start=(ii == 0),
                            stop=(ii == NIC - 1),
                        )
                    # r = 1/Z on partition 64; rank-1 broadcast to [64, 512]
                    rz = attn.tile([128, 512], f32r, tag="rz")
                    with nc.allow_low_precision(reason="1/Z in f32r is fine"):
                        nc.vector.reciprocal(rz[64:65], py[64:65])
                    pb = pa.tile([128, 512], f32, tag="pa")
                    nc.tensor.matmul(
                        pb[:64],
                        ones_t[64:65, 0:64],
                        rz[64:65],
                        start=True,
                        stop=True,
                    )
                    zb = attn.tile([64, 512], f32, tag="zb")
                    nc.vector.tensor_copy(zb, pb[:64])
                    nc.vector.tensor_mul(YT[oc][hb : hb + 64, tsl], py[:64], zb)

        # ---- output projection ----
        with tc.tile_pool(name="wo16", bufs=1) as wo16, \
             tc.tile_pool(name="osb", bufs=3) as osb:
            wot = {}
            for cc in range(NCC):
                for och in range(2):
                    w = wo16.tile([128, 512], f32r, tag=f"wo{cc}_{och}", name=f"wo{cc}_{och}")
                    nc.sync.dma_start(
                        out=w,
                        in_=woT_d[
                            cc * 128 : (cc + 1) * 128, och * 512 : (och + 1) * 512
                        ],
                    )
                    wot[(cc, och)] = w
            for tt in range(8):
                ot = osb.tile([128, C], f32, tag="osb")
                for och in range(2):
                    po = pa.tile([128, 512], f32, tag="pa")
                    for cc in range(NCC):
                        nc.tensor.matmul(
                            po,
                            YT[cc][:, tt * 128 : (tt + 1) * 128],
                            wot[(cc, och)],
                            start=(cc == 0),
                            stop=False,
                        )
                    nc.tensor.matmul(
                        po,
                        ones_t[0:1, 0:128],
                        bo_row[0:1, och * 512 : (och + 1) * 512],
                        start=False,
                        stop=True,
                    )
                    nc.vector.tensor_copy(ot[:, och * 512 : (och + 1) * 512], po)
                nc.sync.dma_start(out=out_d[tt * 128 : (tt + 1) * 128], in_=ot)

    nc.compile()
    return nc


def _get_nc():
    if "nc" not in _CACHE:
        _CACHE["nc"] = _build_nc()
    return _CACHE["nc"]


def _prep_in_maps(x, encoder_output, Wq, bq, Wkv, bkv, Wo, bo):
    f = np.float32
    x = np.asarray(x, f)
    enc = np.asarray(encoder_output, f)
    wqT = np.ascontiguousarray(np.asarray(Wq, f).T)
    wkv = np.asarray(Wkv, f)
    wkT = np.ascontiguousarray(wkv[:C].T)
    wvT = np.ascontiguousarray(wkv[C:].T)
    woT = np.ascontiguousarray(np.asarray(Wo, f).T)
    bq = np.asarray(bq, f)
    bkv = np.asarray(bkv, f)
    bo = np.asarray(bo, f)
    shared = {
        "wqT": wqT, "wkT": wkT, "wvT": wvT, "woT": woT,
        "bq": bq, "bk": np.ascontiguousarray(bkv[:C]),
        "bv": np.ascontiguousarray(bkv[C:]), "bo": bo,
    }
    return [
        dict(shared, x=np.ascontiguousarray(x[b]), enc=np.ascontiguousarray(enc[b]))
        for b in range(x.shape[0])
    ]


def kernel(x, encoder_output, Wq, bq, Wkv, bkv, Wo, bo):
    from concourse.bass_utils import run_bass_kernel_spmd

    nc = _get_nc()
    in_maps = _prep_in_maps(x, encoder_output, Wq, bq, Wkv, bkv, Wo, bo)
    res = run_bass_kernel_spmd(nc, in_maps, list(range(len(in_maps)))).results
    return np.stack([res[b]["out"] for b in range(len(res))]).astype(np.float32)



# revision 16
# speedup vs baseline: 1.4617x; 1.4617x over previous
"""Cross-attention kernel for Trainium2, 8 NeuronCores, data-parallel over batch.

Problem (per batch element b, one per core):
    q  = x_b @ Wq.T + bq                      [T=1024, C=1024]
    kv = enc_b @ Wkv.T + bkv                  [I=576, 2C]
    per head h (H=16, D=64):
        att = softmax((q_h @ k_h.T) / sqrt(D))
        y_h = att @ v_h
    out = y @ Wo.T + bo                       [T, C]

Design notes:
  - One batch element per core (B=8 == n_cores), no collectives.
  - All inputs cast to bf16 on host (weights pre-transposed to [in, out]).
    Tolerance is 2e-2; bf16 keeps us ~1e-3.  PSUM accumulation is f32.
  - x / enc are transposed on-device via the PE (batched up front so the
    identity stays resident).
  - Attention is S^T = K_h @ Q_h^T orientation ([i, t]).  Head pairs share
    an o-chunk: heads 2j/2j+1 live at partitions 0-63 / 64-127 of KT/QT
    chunk j, so their S matmuls (K=64) row-tile onto disjoint PE strips and
    run concurrently.  exp without max-subtraction (scores O(1)).  The
    softmax denominator Z_t rides the AV matmul as a ones column (M=65);
    1/Z via reciprocal_approx_fast on PSUM partition 64, broadcast to
    [64, 512] with a rank-1 PE matmul, applied as an in-place DVE multiply.
  - Biases: bq/bk are per-partition adds fused into the PSUM->SBUF move;
    bv/bo are rank-1 (K=1) matmul accumulates of ones^T (x) bias_row.
"""

import numpy as np

T = 1024
C = 1024
I = 576
H = 16
D = 64
NCC = C // 128          # 8 contraction chunks
NIC = (I + 127) // 128  # 5 i chunks (128,128,128,128,64)
I_CH = [128, 128, 128, 128, 64]
VW = 68                 # per-head column block in V tile: 64 v cols + ones col + pad
SCALE = 1.0 / np.sqrt(D)

_CACHE = {}
DEBUG_TAPS = True


def _build_nc():
    import concourse.bass as bass
    import concourse.bacc as bacc
    import concourse.mybir as mybir
    import concourse.tile as tile
    from contextlib import ExitStack

    f32 = mybir.dt.float32
    f32r = mybir.dt.float32r
    bf16 = mybir.dt.bfloat16
    EXP = mybir.ActivationFunctionType.Exp
    LN = mybir.ActivationFunctionType.Ln

    nc = bacc.Bacc()

    x_d = nc.dram_tensor("x", [T, C], bf16, kind="ExternalInput")
    enc_d = nc.dram_tensor("enc", [I, C], bf16, kind="ExternalInput")
    wqT_d = nc.dram_tensor("wqT", [C, C], bf16, kind="ExternalInput")
    wkT_d = nc.dram_tensor("wkT", [C, C], bf16, kind="ExternalInput")
    wvT_d = nc.dram_tensor("wvT", [C, C], bf16, kind="ExternalInput")
    woT_d = nc.dram_tensor("woT", [C, C], bf16, kind="ExternalInput")
    bq_d = nc.dram_tensor("bq", [C], f32, kind="ExternalInput")
    bk_d = nc.dram_tensor("bk", [C], f32, kind="ExternalInput")
    bv_d = nc.dram_tensor("bv", [C], bf16, kind="ExternalInput")
    bo_d = nc.dram_tensor("bo", [C], bf16, kind="ExternalInput")
    out_d = nc.dram_tensor("out", [T, C], f32, kind="ExternalOutput")
    if DEBUG_TAPS:
        dbg = {
            "dbg_v3ones": nc.dram_tensor("dbg_v3ones", [128, H], bf16, kind="ExternalOutput"),
            "dbg_qt": nc.dram_tensor("dbg_qt", [128, T], bf16, kind="ExternalOutput"),
            "dbg_kt": nc.dram_tensor("dbg_kt", [128, I], bf16, kind="ExternalOutput"),
            "dbg_es": nc.dram_tensor("dbg_es", [128, 512], bf16, kind="ExternalOutput"),
            "dbg_z": nc.dram_tensor("dbg_z", [1, 512], f32, kind="ExternalOutput"),
            "dbg_rz": nc.dram_tensor("dbg_rz", [1, 512], f32, kind="ExternalOutput"),
            "dbg_pb": nc.dram_tensor("dbg_pb", [64, 512], f32, kind="ExternalOutput"),
            "dbg_yt": nc.dram_tensor("dbg_yt", [128, T], bf16, kind="ExternalOutput"),
        }

    with ExitStack() as ctx:
        tc = ctx.enter_context(tile.TileContext(nc))

        misc = ctx.enter_context(tc.tile_pool(name="misc", bufs=1))
        wgt = ctx.enter_context(tc.tile_pool(name="wgt", bufs=1))
        resid = ctx.enter_context(tc.tile_pool(name="resid", bufs=1))

        import ml_dtypes
        ident_d = nc.inline_tensor(
            np.eye(128, dtype=ml_dtypes.bfloat16), name="ident_d"
        )
        onesb_d = nc.inline_tensor(
            np.ones((128, 128), dtype=ml_dtypes.bfloat16), name="onesb_d"
        )
        onesf_d = nc.inline_tensor(
            np.ones((128, 64), dtype=np.float32), name="onesf_d"
        )
        ident = misc.tile([128, 128], bf16)
        nc.sync.dma_start(out=ident, in_=ident_d[:, :])
        ones_b = misc.tile([128, 128], bf16)
        nc.sync.dma_start(out=ones_b, in_=onesb_d[:, :])
        ones_r = misc.tile([128, 64], f32r)
        nc.sync.dma_start(out=ones_r, in_=onesf_d[:, :].bitcast(f32r))
        bq_t = misc.tile([128, NCC], f32)
        nc.sync.dma_start(out=bq_t, in_=bq_d[:].rearrange("(oc p) -> p oc", p=128))
        bk_t = misc.tile([128, NCC], f32)
        nc.sync.dma_start(out=bk_t, in_=bk_d[:].rearrange("(oc p) -> p oc", p=128))
        bv_row = misc.tile([1, C], bf16)
        nc.sync.dma_start(out=bv_row, in_=bv_d[:].unsqueeze(0))
        bo_row = misc.tile([1, C], bf16)
        nc.sync.dma_start(out=bo_row, in_=bo_d[:].unsqueeze(0))

        # resident tensors
        xT = [resid.tile([128, T], bf16, tag=f"xT{i}", name=f"xT{i}") for i in range(NCC)]
        encT = [resid.tile([128, I], bf16, tag=f"encT{i}", name=f"encT{i}") for i in range(NCC)]
        QT = [resid.tile([128, T], bf16, tag=f"QT{i}", name=f"QT{i}") for i in range(NCC)]
        KT = [resid.tile([128, I], bf16, tag=f"KT{i}", name=f"KT{i}") for i in range(NCC)]
        V3 = [resid.tile([128, H, VW], bf16, tag=f"V{i}", name=f"V{i}") for i in range(NIC)]
        YT = [resid.tile([128, T], bf16, tag=f"YT{i}", name=f"YT{i}") for i in range(NCC)]

        # resident weights, DMA'd once up front (bf16: 2KB per partition line)
        wk_t = [wgt.tile([128, C], bf16, tag=f"wk{i}", name=f"wk{i}") for i in range(NCC)]
        wv_t = [wgt.tile([128, C], bf16, tag=f"wv{i}", name=f"wv{i}") for i in range(NCC)]
        wq_t = [wgt.tile([128, C], bf16, tag=f"wq{i}", name=f"wq{i}") for i in range(NCC)]
        wo_t = [wgt.tile([128, C], bf16, tag=f"wo{i}", name=f"wo{i}") for i in range(NCC)]
        for cc in range(NCC):
            nc.sync.dma_start(out=wk_t[cc], in_=wkT_d[cc * 128 : (cc + 1) * 128])
        for cc in range(NCC):
            nc.sync.dma_start(out=wv_t[cc], in_=wvT_d[cc * 128 : (cc + 1) * 128])
        for cc in range(NCC):
            nc.sync.dma_start(out=wq_t[cc], in_=wqT_d[cc * 128 : (cc + 1) * 128])
        for cc in range(NCC):
            nc.sync.dma_start(out=wo_t[cc], in_=woT_d[cc * 128 : (cc + 1) * 128])

        # ones columns for the fused Z row in AV
        for ii in range(NIC):
            nc.sync.dma_start(
                out=V3[ii][:, :, 64:65], in_=onesb_d[:, 0:H].unsqueeze(2)
            )

        # ---- phase A: on-PE transposes of enc and x (identity stays hot) ----
        with tc.tile_pool(name="xin", bufs=3) as xin, \
             tc.tile_pool(name="ptp", bufs=4, space="PSUM") as ptp:
            for ii in range(NIC):
                pi = I_CH[ii]
                e_nat = xin.tile([128, C], bf16, tag="xin")
                nc.sync.dma_start(out=e_nat[:pi], in_=enc_d[ii * 128 : ii * 128 + pi])
                for cc in range(NCC):
                    ps = ptp.tile([128, 128], bf16, tag="pt")
                    nc.tensor.transpose(
                        ps[:128, :pi],
                        e_nat[:pi, cc * 128 : (cc + 1) * 128],
                        ident[:pi, :pi],
                    )
                    eng = nc.vector if cc % 2 == 0 else nc.scalar
                    if cc % 2 == 0:
                        nc.vector.tensor_copy(
                            encT[cc][:, ii * 128 : ii * 128 + pi], ps[:128, :pi]
                        )
                    else:
                        nc.scalar.copy(
                            encT[cc][:, ii * 128 : ii * 128 + pi], ps[:128, :pi]
                        )
            for tt in range(8):
                x_nat = xin.tile([128, C], bf16, tag="xin")
                nc.sync.dma_start(out=x_nat, in_=x_d[tt * 128 : (tt + 1) * 128])
                for cc in range(NCC):
                    ps = ptp.tile([128, 128], bf16, tag="pt")
                    nc.tensor.transpose(
                        ps, x_nat[:, cc * 128 : (cc + 1) * 128], ident
                    )
                    if cc % 2 == 0:
                        nc.vector.tensor_copy(
                            xT[cc][:, tt * 128 : (tt + 1) * 128], ps
                        )
                    else:
                        nc.scalar.copy(
                            xT[cc][:, tt * 128 : (tt + 1) * 128], ps
                        )

        # ---- phases B/C/D: K, V, Q projections ----
        with tc.tile_pool(name="pkp", bufs=4, space="PSUM") as pkp, \
             tc.tile_pool(name="pjp", bufs=4, space="PSUM") as pjp:

            # K^T[o, i] = WkT.T @ encT, i split 288+288
            for oc in range(NCC):
                pk = [pkp.tile([128, 288], f32, tag="pk", name=f"pk{_}") for _ in range(2)]
                for cc in range(NCC):
                    for ih in range(2):
                        nc.tensor.matmul(
                            pk[ih],
                            wk_t[cc][:, oc * 128 : (oc + 1) * 128],
                            encT[cc][:, ih * 288 : (ih + 1) * 288],
                            start=(cc == 0),
                            stop=(cc == NCC - 1),
                        )
                for ih in range(2):
                    if ih == 0:
                        nc.vector.tensor_scalar_add(
                            KT[oc][:, ih * 288 : (ih + 1) * 288],
                            pk[ih],
                            bk_t[:, oc : oc + 1],
                        )
                    else:
                        nc.scalar.add(
                            KT[oc][:, ih * 288 : (ih + 1) * 288],
                            pk[ih],
                            bk_t[:, oc : oc + 1],
                        )

            # V[i, o] = encT.T @ WvT (+ bv rank-1), into [128, H, VW] layout
            for ii in range(NIC):
                pi = I_CH[ii]
                for och in range(2):
                    pv = pjp.tile([128, 512], f32, tag="pj")
                    for cc in range(NCC):
                        nc.tensor.matmul(
                            pv[:pi],
                            encT[cc][:, ii * 128 : ii * 128 + pi],
                            wv_t[cc][:, och * 512 : (och + 1) * 512],
                            start=(cc == 0),
                            stop=False,
                        )
                    nc.tensor.matmul(
                        pv[:pi],
                        ones_b[0:1, :pi],
                        bv_row[0:1, och * 512 : (och + 1) * 512],
                        start=False,
                        stop=True,
                    )
                    dst = V3[ii][:pi, och * 8 : och * 8 + 8, 0:64]
                    nc.vector.tensor_copy(
                        dst, pv[:pi].rearrange("p (h d) -> p h d", d=64)
                    )

            # Q^T[o, t] = WqT.T @ xT
            for tch in range(2):
                for oc in range(NCC):
                    pq = pjp.tile([128, 512], f32, tag="pj")
                    for cc in range(NCC):
                        nc.tensor.matmul(
                            pq,
                            wq_t[cc][:, oc * 128 : (oc + 1) * 128],
                            xT[cc][:, tch * 512 : (tch + 1) * 512],
                            start=(cc == 0),
                            stop=(cc == NCC - 1),
                        )
                    if oc % 2 == 0:
                        nc.vector.tensor_scalar_add(
                            QT[oc][:, tch * 512 : (tch + 1) * 512],
                            pq,
                            bq_t[:, oc : oc + 1],
                        )
                    else:
                        nc.scalar.add(
                            QT[oc][:, tch * 512 : (tch + 1) * 512],
                            pq,
                            bq_t[:, oc : oc + 1],
                        )

        # ---- phase E: attention ----
        # Software-pipelined: S(hp+1) is emitted before AV(hp) so the PE has
        # work while the scalar engine chews through exp(hp).
        with tc.tile_pool(name="spp", bufs=3, space="PSUM") as spp, \
             tc.tile_pool(name="pyp", bufs=3, space="PSUM") as pyp, \
             tc.tile_pool(name="pbp", bufs=2, space="PSUM") as pbp, \
             tc.tile_pool(name="esp", bufs=20) as esp, \
             tc.tile_pool(name="rzp", bufs=4) as rzp, \
             tc.tile_pool(name="ybp", bufs=4) as ybp:

            def emit_S(hp, tch):
                """S^T chunks for head pair hp -> exp -> es tiles (bf16)."""
                oc = hp
                tsl = slice(tch * 512, (tch + 1) * 512)
                es = []
                for ii in range(NIC):
                    pi = I_CH[ii]
                    isl = slice(ii * 128, ii * 128 + pi)
                    sA = spp.tile([128, 512], f32, tag="sp")
                    sB = spp.tile([128, 512], f32, tag="sp")
                    # row-tiled pair: strips 0-63 / 64-127 run concurrently
                    nc.tensor.matmul(
                        sA[:pi], KT[oc][0:64, isl], QT[oc][0:64, tsl],
                        start=True, stop=True,
                    )
                    nc.tensor.matmul(
                        sB[:pi], KT[oc][64:128, isl], QT[oc][64:128, tsl],
                        start=True, stop=True,
                    )
                    eA = esp.tile([128, 512], bf16, tag="es")
                    eB = esp.tile([128, 512], bf16, tag="es")
                    nc.scalar.activation(eA[:pi], sA[:pi], EXP, scale=float(SCALE))
                    nc.scalar.activation(eB[:pi], sB[:pi], EXP, scale=float(SCALE))
                    if DEBUG_TAPS and hp == 0 and tch == 0 and ii == 0:
                        nc.sync.dma_start(out=dbg["dbg_es"][:, :], in_=eA)
                    es.append((eA, eB))
                return es

            def emit_AV(hp, tch, es):
                """AV + fused Z row + 1/Z broadcast + normalized write to YT."""
                oc = hp
                tsl = slice(tch * 512, (tch + 1) * 512)
                for half in range(2):
                    h = 2 * hp + half
                    hb = half * 64
                    py = pyp.tile([65, 512], f32, tag="py")
                    for ii in range(NIC):
                        pi = I_CH[ii]
                        nc.tensor.matmul(
                            py[:65],
                            V3[ii][:pi, h, 0:65],
                            es[ii][half][:pi],
                            start=(ii == 0),
                            stop=(ii == NIC - 1),
                        )
                    # 1/Z = exp(-ln Z) on the scalar engine (PSUM-capable,
                    # standard table functions), emitted bf16 for the
                    # rank-1 broadcast matmul.
                    lnz = rzp.tile([65, 512], f32, tag="rz")
                    nc.scalar.activation(lnz[64:65], py[64:65], LN)
                    rzb = rzp.tile([65, 512], bf16, tag="rzb")
                    nc.scalar.activation(rzb[64:65], lnz[64:65], EXP, scale=-1.0)
                    pb = pbp.tile([64, 512], f32, tag="pb")
                    nc.tensor.matmul(
                        pb[:64],
                        ones_b[64:65, 0:64],
                        rzb[64:65],
                        start=True, stop=True,
                    )
                    # unnormalized y -> scratch (scalar engine), then *= bcast(1/Z)
                    yb = ybp.tile([64, 512], bf16, tag="yb")
                    nc.scalar.copy(yb, py[0:64])
                    if DEBUG_TAPS and hp == 0 and tch == 0 and half == 0:
                        zs = rzp.tile([65, 512], f32, tag="zs")
                        nc.vector.tensor_copy(zs[64:65], py[64:65])
                        nc.sync.dma_start(out=dbg["dbg_z"][:, :], in_=zs[64:65])
                        nc.sync.dma_start(out=dbg["dbg_rz"][:, :], in_=lnz[64:65])
                        pbs = rzp.tile([64, 512], f32, tag="pbs")
                        nc.vector.tensor_copy(pbs, pb[:64])
                        nc.sync.dma_start(out=dbg["dbg_pb"][:, :], in_=pbs)
                    nc.vector.tensor_mul(
                        YT[oc][hb : hb + 64, tsl],
                        yb,
                        pb[:64],
                    )

            for tch in range(2):
                es_prev = None
                for hp in range(NCC):
                    es_cur = emit_S(hp, tch)
                    if es_prev is not None:
                        emit_AV(hp - 1, tch, es_prev)
                    es_prev = es_cur
                emit_AV(NCC - 1, tch, es_prev)

        if DEBUG_TAPS:
            nc.sync.dma_start(
                out=dbg["dbg_v3ones"][:, :].unsqueeze(2), in_=V3[0][:, :, 64:65]
            )
            nc.sync.dma_start(out=dbg["dbg_qt"][:, :], in_=QT[0])
            nc.sync.dma_start(out=dbg["dbg_kt"][:, :], in_=KT[0])
            nc.sync.dma_start(out=dbg["dbg_yt"][:, :], in_=YT[0])

        # ---- phase G: output projection ----
        with tc.tile_pool(name="pop", bufs=4, space="PSUM") as pop, \
             tc.tile_pool(name="osb", bufs=2) as osb:
            for tt in range(8):
                ot = osb.tile([128, C], f32, tag="osb")
                for och in range(2):
                    po = pop.tile([128, 512], f32, tag="po")
                    for cc in range(NCC):
                        nc.tensor.matmul(
                            po,
                            YT[cc][:, tt * 128 : (tt + 1) * 128],
                            wo_t[cc][:, och * 512 : (och + 1) * 512],
                            start=(cc == 0),
                            stop=False,
                        )
                    nc.tensor.matmul(
                        po,
                        ones_b[0:1, 0:128],
                        bo_row[0:1, och * 512 : (och + 1) * 512],
                        start=False,
                        stop=True,
                    )
                    if och == 0:
                        nc.vector.tensor_copy(ot[:, och * 512 : (och + 1) * 512], po)
                    else:
                        nc.scalar.copy(ot[:, och * 512 : (och + 1) * 512], po)
                nc.sync.dma_start(out=out_d[tt * 128 : (tt + 1) * 128], in_=ot)

    nc.compile()
    return nc


def _get_nc():
    if "nc" not in _CACHE:
        _CACHE["nc"] = _build_nc()
    return _CACHE["nc"]


def _prep_in_maps(x, encoder_output, Wq, bq, Wkv, bkv, Wo, bo):
    import ml_dtypes

    bf = ml_dtypes.bfloat16
    f = np.float32
    x = np.asarray(x, f)
    enc = np.asarray(encoder_output, f)
    wqT = np.ascontiguousarray(np.asarray(Wq, f).T).astype(bf)
    wkv = np.asarray(Wkv, f)
    wkT = np.ascontiguousarray(wkv[:C].T).astype(bf)
    wvT = np.ascontiguousarray(wkv[C:].T).astype(bf)
    woT = np.ascontiguousarray(np.asarray(Wo, f).T).astype(bf)
    bkv = np.asarray(bkv, f)
    shared = {
        "wqT": wqT, "wkT": wkT, "wvT": wvT, "woT": woT,
        "bq": np.asarray(bq, f),
        "bk": np.ascontiguousarray(bkv[:C]),
        "bv": np.ascontiguousarray(bkv[C:]).astype(bf),
        "bo": np.asarray(bo, f).astype(bf),
    }
    return [
        dict(
            shared,
            x=np.ascontiguousarray(x[b]).astype(bf),
            enc=np.ascontiguousarray(enc[b]).astype(bf),
        )
        for b in range(x.shape[0])
    ]


def kernel(x, encoder_output, Wq, bq, Wkv, bkv, Wo, bo):
    from concourse.bass_utils import run_bass_kernel_spmd

    nc = _get_nc()
    in_maps = _prep_in_maps(x, encoder_output, Wq, bq, Wkv, bkv, Wo, bo)
    res = run_bass_kernel_spmd(nc, in_maps, list(range(len(in_maps)))).results
    return np.stack([res[b]["out"] for b in range(len(res))]).astype(np.float32)


# revision 20
# speedup vs baseline: 1.9043x; 1.3028x over previous
"""Cross-attention kernel for Trainium2, 8 NeuronCores, data-parallel over batch.

Problem (per batch element b, one per core):
    q  = x_b @ Wq.T + bq                      [T=1024, C=1024]
    kv = enc_b @ Wkv.T + bkv                  [I=576, 2C]
    per head h (H=16, D=64):
        att = softmax((q_h @ k_h.T) / sqrt(D))
        y_h = att @ v_h
    out = y @ Wo.T + bo                       [T, C]

Design notes:
  - One batch element per core (B=8 == n_cores), no collectives.
  - All inputs cast to bf16 on host (weights pre-transposed to [in, out]).
    Tolerance is 2e-2; bf16 keeps us ~1e-3.  PSUM accumulation is f32.
  - x / enc are transposed on-device via the PE (batched up front so the
    identity stays resident).
  - Attention is S^T = K_h @ Q_h^T orientation ([i, t]).  Head pairs share
    an o-chunk: heads 2j/2j+1 live at partitions 0-63 / 64-127 of KT/QT
    chunk j, so their S matmuls (K=64) row-tile onto disjoint PE strips and
    run concurrently.  exp without max-subtraction (scores O(1)).  The
    softmax denominator Z_t rides the AV matmul as a ones column (M=65);
    1/Z via reciprocal_approx_fast on PSUM partition 64, broadcast to
    [64, 512] with a rank-1 PE matmul, applied as an in-place DVE multiply.
  - Biases: bq/bk are per-partition adds fused into the PSUM->SBUF move;
    bv/bo are rank-1 (K=1) matmul accumulates of ones^T (x) bias_row.
"""

import numpy as np

T = 1024
C = 1024
I = 576
H = 16
D = 64
NCC = C // 128          # 8 contraction chunks
NIC = (I + 127) // 128  # 5 i chunks (128,128,128,128,64)
I_CH = [128, 128, 128, 128, 64]
VW = 68                 # per-head column block in V tile: 64 v cols + ones col + pad
SCALE = 1.0 / np.sqrt(D)

_CACHE = {}
DEBUG_TAPS = True


def _build_nc():
    import concourse.bass as bass
    import concourse.bacc as bacc
    import concourse.mybir as mybir
    import concourse.tile as tile
    from contextlib import ExitStack

    f32 = mybir.dt.float32
    f32r = mybir.dt.float32r
    bf16 = mybir.dt.bfloat16
    EXP = mybir.ActivationFunctionType.Exp
    LN = mybir.ActivationFunctionType.Ln

    nc = bacc.Bacc()

    x_d = nc.dram_tensor("x", [T, C], bf16, kind="ExternalInput")
    enc_d = nc.dram_tensor("enc", [I, C], bf16, kind="ExternalInput")
    wqT_d = nc.dram_tensor("wqT", [C, C], bf16, kind="ExternalInput")
    wkT_d = nc.dram_tensor("wkT", [C, C], bf16, kind="ExternalInput")
    wvT_d = nc.dram_tensor("wvT", [C, C], bf16, kind="ExternalInput")
    woT_d = nc.dram_tensor("woT", [C, C], bf16, kind="ExternalInput")
    bq_d = nc.dram_tensor("bq", [C], f32, kind="ExternalInput")
    bk_d = nc.dram_tensor("bk", [C], f32, kind="ExternalInput")
    bv_d = nc.dram_tensor("bv", [C], bf16, kind="ExternalInput")
    bo_d = nc.dram_tensor("bo", [C], bf16, kind="ExternalInput")
    out_d = nc.dram_tensor("out", [T, C], f32, kind="ExternalOutput")
    if DEBUG_TAPS:
        dbg = {
            "dbg_es": nc.dram_tensor("dbg_es", [128, 512], bf16, kind="ExternalOutput"),
            "dbg_rzb": nc.dram_tensor("dbg_rzb", [16, 1024], bf16, kind="ExternalOutput"),
            "dbg_yt": nc.dram_tensor("dbg_yt", [128, T], bf16, kind="ExternalOutput"),
        }

    with ExitStack() as ctx:
        tc = ctx.enter_context(tile.TileContext(nc))

        misc = ctx.enter_context(tc.tile_pool(name="misc", bufs=1))
        wgt = ctx.enter_context(tc.tile_pool(name="wgt", bufs=1))
        resid = ctx.enter_context(tc.tile_pool(name="resid", bufs=1))

        import ml_dtypes
        ident_d = nc.inline_tensor(
            np.eye(128, dtype=ml_dtypes.bfloat16), name="ident_d"
        )
        onesb_d = nc.inline_tensor(
            np.ones((128, 128), dtype=ml_dtypes.bfloat16), name="onesb_d"
        )
        onesf_d = nc.inline_tensor(
            np.ones((128, 64), dtype=np.float32), name="onesf_d"
        )
        ident = misc.tile([128, 128], bf16)
        nc.sync.dma_start(out=ident, in_=ident_d[:, :])
        ones_b = misc.tile([128, 128], bf16)
        nc.sync.dma_start(out=ones_b, in_=onesb_d[:, :])
        identf_d = nc.inline_tensor(np.eye(128, dtype=np.float32), name="identf_d")
        ident_r = misc.tile([128, 128], f32r)
        nc.sync.dma_start(out=ident_r, in_=identf_d[:, :].bitcast(f32r))
        sel_np = np.kron(np.eye(16), np.ones((1, 64))).astype(ml_dtypes.bfloat16)
        sel_d = nc.inline_tensor(sel_np, name="sel_d")
        sel_t = misc.tile([16, 1024], bf16)
        nc.sync.dma_start(out=sel_t, in_=sel_d[:, :])
        bq_t = misc.tile([128, NCC], f32)
        nc.sync.dma_start(out=bq_t, in_=bq_d[:].rearrange("(oc p) -> p oc", p=128))
        bk_t = misc.tile([128, NCC], f32)
        nc.sync.dma_start(out=bk_t, in_=bk_d[:].rearrange("(oc p) -> p oc", p=128))
        bv_row = misc.tile([1, C], bf16)
        nc.sync.dma_start(out=bv_row, in_=bv_d[:].unsqueeze(0))
        bo_row = misc.tile([1, C], bf16)
        nc.sync.dma_start(out=bo_row, in_=bo_d[:].unsqueeze(0))

        # resident tensors
        xT = [resid.tile([128, T], bf16, tag=f"xT{i}", name=f"xT{i}") for i in range(NCC)]
        encT = [resid.tile([128, I], bf16, tag=f"encT{i}", name=f"encT{i}") for i in range(NCC)]
        QT = [resid.tile([128, T], bf16, tag=f"QT{i}", name=f"QT{i}") for i in range(NCC)]
        KT = [resid.tile([128, I], bf16, tag=f"KT{i}", name=f"KT{i}") for i in range(NCC)]
        V3 = [resid.tile([128, H, VW], bf16, tag=f"V{i}", name=f"V{i}") for i in range(NIC)]
        YT = [resid.tile([128, T], bf16, tag=f"YT{i}", name=f"YT{i}") for i in range(NCC)]

        # resident weights, DMA'd once up front (bf16: 2KB per partition line)
        wk_t = [wgt.tile([128, C], bf16, tag=f"wk{i}", name=f"wk{i}") for i in range(NCC)]
        wv_t = [wgt.tile([128, C], bf16, tag=f"wv{i}", name=f"wv{i}") for i in range(NCC)]
        wq_t = [wgt.tile([128, C], bf16, tag=f"wq{i}", name=f"wq{i}") for i in range(NCC)]
        wo_t = [wgt.tile([128, C], bf16, tag=f"wo{i}", name=f"wo{i}") for i in range(NCC)]
        # ones columns for the fused Z row in AV
        for ii in range(NIC):
            nc.sync.dma_start(
                out=V3[ii][:, :, 64:65], in_=onesb_d[:, 0:H].unsqueeze(2)
            )

        # ---- phase A: on-PE transposes of enc and x (identity stays hot) ----
        with tc.tile_pool(name="xin", bufs=3) as xin, \
             tc.tile_pool(name="ptp", bufs=4, space="PSUM") as ptp:
            # activations first so transposes start immediately; weights after
            # (they are consumed later and stream in under the transposes).
            e_nats = []
            for ii in range(NIC):
                pi = I_CH[ii]
                e_nat = xin.tile([128, C], bf16, tag="xin", bufs=13, name=f"e_nat{ii}")
                nc.sync.dma_start(out=e_nat[:pi], in_=enc_d[ii * 128 : ii * 128 + pi])
                e_nats.append(e_nat)
            x_nats = []
            for tt in range(8):
                x_nat = xin.tile([128, C], bf16, tag="xin", bufs=13, name=f"x_nat{tt}")
                nc.sync.dma_start(out=x_nat, in_=x_d[tt * 128 : (tt + 1) * 128])
                x_nats.append(x_nat)
            for cc in range(NCC):
                nc.sync.dma_start(out=wk_t[cc], in_=wkT_d[cc * 128 : (cc + 1) * 128])
            for cc in range(NCC):
                nc.sync.dma_start(out=wv_t[cc], in_=wvT_d[cc * 128 : (cc + 1) * 128])
            for cc in range(NCC):
                nc.sync.dma_start(out=wq_t[cc], in_=wqT_d[cc * 128 : (cc + 1) * 128])
            for cc in range(NCC):
                nc.sync.dma_start(out=wo_t[cc], in_=woT_d[cc * 128 : (cc + 1) * 128])
            for ii in range(NIC):
                pi = I_CH[ii]
                e_nat = e_nats[ii]
                for cc in range(NCC):
                    ps = ptp.tile([128, 128], bf16, tag="pt")
                    nc.tensor.transpose(
                        ps[:128, :pi],
                        e_nat[:pi, cc * 128 : (cc + 1) * 128],
                        ident[:pi, :pi],
                    )
                    eng = nc.vector if cc % 2 == 0 else nc.scalar
                    if cc % 2 == 0:
                        nc.vector.tensor_copy(
                            encT[cc][:, ii * 128 : ii * 128 + pi], ps[:128, :pi]
                        )
                    else:
                        nc.scalar.copy(
                            encT[cc][:, ii * 128 : ii * 128 + pi], ps[:128, :pi]
                        )
            for tt in range(8):
                x_nat = x_nats[tt]
                for cc in range(NCC):
                    ps = ptp.tile([128, 128], bf16, tag="pt")
                    nc.tensor.transpose(
                        ps, x_nat[:, cc * 128 : (cc + 1) * 128], ident
                    )
                    if cc % 2 == 0:
                        nc.vector.tensor_copy(
                            xT[cc][:, tt * 128 : (tt + 1) * 128], ps
                        )
                    else:
                        nc.scalar.copy(
                            xT[cc][:, tt * 128 : (tt + 1) * 128], ps
                        )

        # ---- phases B/C/D: K, V, Q projections ----
        with tc.tile_pool(name="pkp", bufs=4, space="PSUM") as pkp, \
             tc.tile_pool(name="pjp", bufs=4, space="PSUM") as pjp:

            # K^T[o, i] = WkT.T @ encT, i split 288+288
            for oc in range(NCC):
                pk = [pkp.tile([128, 288], f32, tag="pk", name=f"pk{_}") for _ in range(2)]
                for cc in range(NCC):
                    for ih in range(2):
                        nc.tensor.matmul(
                            pk[ih],
                            wk_t[cc][:, oc * 128 : (oc + 1) * 128],
                            encT[cc][:, ih * 288 : (ih + 1) * 288],
                            start=(cc == 0),
                            stop=(cc == NCC - 1),
                        )
                for ih in range(2):
                    if ih == 0:
                        nc.vector.tensor_scalar_add(
                            KT[oc][:, ih * 288 : (ih + 1) * 288],
                            pk[ih],
                            bk_t[:, oc : oc + 1],
                        )
                    else:
                        nc.scalar.add(
                            KT[oc][:, ih * 288 : (ih + 1) * 288],
                            pk[ih],
                            bk_t[:, oc : oc + 1],
                        )

            # V[i, o] = encT.T @ WvT (+ bv rank-1), into [128, H, VW] layout
            for ii in range(NIC):
                pi = I_CH[ii]
                for och in range(2):
                    pv = pjp.tile([128, 512], f32, tag="pj")
                    for cc in range(NCC):
                        nc.tensor.matmul(
                            pv[:pi],
                            encT[cc][:, ii * 128 : ii * 128 + pi],
                            wv_t[cc][:, och * 512 : (och + 1) * 512],
                            start=(cc == 0),
                            stop=False,
                        )
                    nc.tensor.matmul(
                        pv[:pi],
                        ones_b[0:1, :pi],
                        bv_row[0:1, och * 512 : (och + 1) * 512],
                        start=False,
                        stop=True,
                    )
                    dst = V3[ii][:pi, och * 8 : och * 8 + 8, 0:64]
                    nc.vector.tensor_copy(
                        dst, pv[:pi].rearrange("p (h d) -> p h d", d=64)
                    )

            # Q^T[o, t] = WqT.T @ xT
            for tch in range(2):
                for oc in range(NCC):
                    pq = pjp.tile([128, 512], f32, tag="pj")
                    for cc in range(NCC):
                        nc.tensor.matmul(
                            pq,
                            wq_t[cc][:, oc * 128 : (oc + 1) * 128],
                            xT[cc][:, tch * 512 : (tch + 1) * 512],
                            start=(cc == 0),
                            stop=(cc == NCC - 1),
                        )
                    if oc % 2 == 0:
                        nc.vector.tensor_scalar_add(
                            QT[oc][:, tch * 512 : (tch + 1) * 512],
                            pq,
                            bq_t[:, oc : oc + 1],
                        )
                    else:
                        nc.scalar.add(
                            QT[oc][:, tch * 512 : (tch + 1) * 512],
                            pq,
                            bq_t[:, oc : oc + 1],
                        )

        # ---- phase E: attention ----
        # Per head pair hp (heads 2hp/2hp+1 at partitions 0-63/64-127 of
        # KT/QT chunk hp), both t-halves together.  S pairs row-tile onto
        # disjoint PE strips; S lands in [128,1024] two-bank PSUM tiles so
        # one exp covers both t-halves.  Unnormalized y goes straight to YT;
        # Z rows are PE-gathered into one [16,1024] PSUM tile (row = head)
        # so 1/Z for all heads is a single Ln+Exp pair at the tail, then a
        # selector-matmul broadcast and an in-place multiply fix up YT while
        # the output projection runs per t-half.
        with tc.tile_pool(name="pzp", bufs=1, space="PSUM") as pzp:
            pz = pzp.tile([16, 1024], f32, tag="pz", name="pz")
            with tc.tile_pool(name="spp", bufs=2, space="PSUM") as spp, \
                 tc.tile_pool(name="pyp", bufs=2, space="PSUM") as pyp, \
                 tc.tile_pool(name="esp", bufs=20) as esp, \
                 tc.tile_pool(name="zrp", bufs=3) as zrp:

                def emit_S(hp):
                    oc = hp
                    es = []
                    for ii in range(NIC):
                        pi = I_CH[ii]
                        isl = slice(ii * 128, ii * 128 + pi)
                        sA = spp.tile([128, 1024], f32, tag="sp")
                        sB = spp.tile([128, 1024], f32, tag="sp")
                        for tch in range(2):
                            tsl = slice(tch * 512, (tch + 1) * 512)
                            nc.tensor.matmul(
                                sA[:pi, tsl], KT[oc][0:64, isl], QT[oc][0:64, tsl],
                                start=True, stop=True,
                            )
                            nc.tensor.matmul(
                                sB[:pi, tsl], KT[oc][64:128, isl], QT[oc][64:128, tsl],
                                start=True, stop=True,
                            )
                        eA = esp.tile([128, 1024], bf16, tag="es")
                        eB = esp.tile([128, 1024], bf16, tag="es")
                        nc.scalar.activation(eA[:pi], sA[:pi], EXP, scale=float(SCALE))
                        nc.scalar.activation(eB[:pi], sB[:pi], EXP, scale=float(SCALE))
                        if DEBUG_TAPS and hp == 0 and ii == 0:
                            nc.sync.dma_start(out=dbg["dbg_es"][:, :], in_=eA[:, 0:512])
                        es.append((eA, eB))
                    return es

                def emit_AV(hp, es):
                    oc = hp
                    for half in range(2):
                        h = 2 * hp + half
                        hb = half * 64
                        for tch in range(2):
                            tsl = slice(tch * 512, (tch + 1) * 512)
                            py = pyp.tile([65, 512], f32, tag="py")
                            for ii in range(NIC):
                                pi = I_CH[ii]
                                nc.tensor.matmul(
                                    py[:65],
                                    V3[ii][:pi, h, 0:65],
                                    es[ii][half][:pi, tsl],
                                    start=(ii == 0),
                                    stop=(ii == NIC - 1),
                                )
                            # unnormalized y -> YT (normalized in place later)
                            nc.vector.tensor_copy(YT[oc][hb : hb + 64, tsl], py[0:64])
                            # Z row -> SBUF (f32r-rounded), PE-gather to pz row h
                            zrow = zrp.tile([65, 512], f32r, tag="zrow")
                            nc.vector.tensor_copy(zrow[64:65], py[64:65])
                            nc.tensor.matmul(
                                pz[0:16, tsl],
                                ident_r[64:65, 64 - h : 80 - h],
                                zrow[64:65],
                                start=(h == 0),
                                stop=(h == 15),
                            )

                es_prev = None
                for hp in range(NCC):
                    es_cur = emit_S(hp)
                    if es_prev is not None:
                        emit_AV(hp - 1, es_prev)
                    es_prev = es_cur
                emit_AV(NCC - 1, es_prev)

            # ---- tail: 1/Z for all heads, broadcast, in-place normalize,
            # ---- interleaved with the output projection per t-half
            with tc.tile_pool(name="pbp", bufs=2, space="PSUM") as pbp, \
                 tc.tile_pool(name="pop", bufs=4, space="PSUM") as pop, \
                 tc.tile_pool(name="rzt", bufs=1) as rzt, \
                 tc.tile_pool(name="osb", bufs=2) as osb:
                rzl = rzt.tile([16, 1024], f32, tag="rzl", name="rzl")
                nc.scalar.activation(rzl, pz[0:16, :], LN)
                rzb = rzt.tile([16, 1024], bf16, tag="rzb", name="rzb")
                nc.scalar.activation(rzb, rzl, EXP, scale=-1.0)
                if DEBUG_TAPS:
                    nc.sync.dma_start(out=dbg["dbg_rzb"][:, :], in_=rzb)

                for tch in range(2):
                    tsl = slice(tch * 512, (tch + 1) * 512)
                    for h in range(H):
                        oc = h // 2
                        hb = (h % 2) * 64
                        pb = pbp.tile([64, 512], f32, tag="pb")
                        nc.tensor.matmul(
                            pb[:64],
                            sel_t[0:16, h * 64 : (h + 1) * 64],
                            rzb[0:16, tsl],
                            start=True, stop=True,
                        )
                        nc.vector.tensor_mul(
                            YT[oc][hb : hb + 64, tsl],
                            YT[oc][hb : hb + 64, tsl],
                            pb[:64],
                        )
                    if DEBUG_TAPS and tch == 0:
                        nc.sync.dma_start(out=dbg["dbg_yt"][:, :], in_=YT[0])
                    for tt in range(tch * 4, tch * 4 + 4):
                        ot = osb.tile([128, C], f32, tag="osb")
                        for och in range(2):
                            po = pop.tile([128, 512], f32, tag="po")
                            for cc in range(NCC):
                                nc.tensor.matmul(
                                    po,
                                    YT[cc][:, tt * 128 : (tt + 1) * 128],
                                    wo_t[cc][:, och * 512 : (och + 1) * 512],
                                    start=(cc == 0),
                                    stop=False,
                                )
                            nc.tensor.matmul(
                                po,
                                ones_b[0:1, 0:128],
                                bo_row[0:1, och * 512 : (och + 1) * 512],
                                start=False,
                                stop=True,
                            )
                            if och == 0:
                                nc.vector.tensor_copy(ot[:, och * 512 : (och + 1) * 512], po)
                            else:
                                nc.scalar.copy(ot[:, och * 512 : (och + 1) * 512], po)
                        nc.sync.dma_start(out=out_d[tt * 128 : (tt + 1) * 128], in_=ot)

    nc.compile()
    return nc


def _get_nc():
    if "nc" not in _CACHE:
        _CACHE["nc"] = _build_nc()
    return _CACHE["nc"]


def _prep_in_maps(x, encoder_output, Wq, bq, Wkv, bkv, Wo, bo):
    import ml_dtypes

    bf = ml_dtypes.bfloat16
    f = np.float32
    x = np.asarray(x, f)
    enc = np.asarray(encoder_output, f)
    wqT = np.ascontiguousarray(np.asarray(Wq, f).T).astype(bf)
    wkv = np.asarray(Wkv, f)
    wkT = np.ascontiguousarray(wkv[:C].T).astype(bf)
    wvT = np.ascontiguousarray(wkv[C:].T).astype(bf)
    woT = np.ascontiguousarray(np.asarray(Wo, f).T).astype(bf)
    bkv = np.asarray(bkv, f)
    shared = {
        "wqT": wqT, "wkT": wkT, "wvT": wvT, "woT": woT,
        "bq": np.asarray(bq, f),
        "bk": np.ascontiguousarray(bkv[:C]),
        "bv": np.ascontiguousarray(bkv[C:]).astype(bf),
        "bo": np.asarray(bo, f).astype(bf),
    }
    return [
        dict(
            shared,
            x=np.ascontiguousarray(x[b]).astype(bf),
            enc=np.ascontiguousarray(enc[b]).astype(bf),
        )
        for b in range(x.shape[0])
    ]


def kernel(x, encoder_output, Wq, bq, Wkv, bkv, Wo, bo):
    from concourse.bass_utils import run_bass_kernel_spmd

    nc = _get_nc()
    in_maps = _prep_in_maps(x, encoder_output, Wq, bq, Wkv, bkv, Wo, bo)
    res = run_bass_kernel_spmd(nc, in_maps, list(range(len(in_maps)))).results
    return np.stack([res[b]["out"] for b in range(len(res))]).astype(np.float32)
